# revision 73
# baseline (speedup 1.0000x reference)
"""CAM+SE module kernel for Trainium2, data-parallel over batch across 8 cores.

Reference computation (per sample):
    q = x.reshape(C, HW)
    energy = q @ q.T                      # C x C, symmetric
    att = softmax(max(energy) - energy)   # row-wise; == exp(mn_c - e) / Z_c
    ch_out = att @ q
    se = sigmoid(relu(mean_hw(x) @ W1 + b1) @ W2 + b2)
    out = gamma * (ch_out * se[:, None]) + x

Key layout tricks:
  - energy is symmetric, so softmax stats (row-min mn, Z) are computed in
    natural [c, d] layout; only the unnormalized P = exp(mn - e) needs
    transposing for the second matmul, and the 1/Z + se + gamma factors fold
    into one per-partition scale applied after matmul 2.
  - SE branch runs wholly in column layout ([*, 1] tiles), zero transposes.
  - The attention branch (both big matmuls + PE-transposes) runs in bf16
    (host-cast input), f32 accumulation; softmax stats and the residual path
    stay f32, and the residual add reads an exact f32 copy of x, so the
    gamma*(...)+x output is bit-exact in the gamma=0 regime and standard
    mixed-precision otherwise.
"""

import numpy as np

B, C, H, W = 16, 512, 64, 64
HW = H * W
NCORES = 8
BS = B // NCORES          # samples per core
CT = C // 128             # 4 c-tiles
NT = HW // 128            # 32 n-tiles
NCH = HW // 512           # 8 chunks for matmul2 / output
R = C // 8                # 64

_BUILT = None
LAST_RESULTS = None
TRACE = False
# tunables (A/B tested against the timeline cost model)
CFG = {
    "qt_bufs": 4,       # qT ring depth
    "out_eng": "sync",  # engine issuing output DMAs
    "qb_bufs": 1,       # bf16 x-copy ring depth
    "st_bufs": 5,
}


def _build():
    global _BUILT
    if _BUILT is not None:
        return _BUILT

    import concourse.bacc as bacc
    import concourse.mybir as mybir
    import concourse.tile as tile
    from concourse.masks import make_identity

    f32 = mybir.dt.float32
    bf16 = mybir.dt.bfloat16
    ALU = mybir.AluOpType
    ACT = mybir.ActivationFunctionType

    nc = bacc.Bacc(
        "TRN2",
        target_bir_lowering=False,
        debug=False,
        enable_asserts=False,
        num_devices=NCORES,
    )

    # x is loaded once as exact f32 (residual + SE); the attention branch
    # uses an on-chip bf16 copy produced by the otherwise-idle GPSIMD engine.
    x_d = nc.dram_tensor("x", (BS, C, HW), f32, kind="ExternalInput").ap()
    w1_d = nc.dram_tensor("w1", (C, R), f32, kind="ExternalInput").ap()
    b1_d = nc.dram_tensor("b1", (R, 1), f32, kind="ExternalInput").ap()
    w2_d = nc.dram_tensor("w2", (R, C), f32, kind="ExternalInput").ap()
    b2_d = nc.dram_tensor("b2", (C, 1), f32, kind="ExternalInput").ap()
    g_d = nc.dram_tensor("gam", (1, 1), f32, kind="ExternalInput").ap()
    out_d = nc.dram_tensor("out", (BS, C, HW), f32, kind="ExternalOutput").ap()

    with tile.TileContext(nc) as tc:
        with (
            tc.tile_pool(name="qpool", bufs=2) as qpool,
            tc.tile_pool(name="qtpool", bufs=CFG["qt_bufs"]) as qtpool,
            tc.tile_pool(name="ppool", bufs=1) as ppool,
            tc.tile_pool(name="ptpool", bufs=2) as ptpool,
            tc.tile_pool(name="stpool", bufs=4) as stpool,
            tc.tile_pool(name="stat", bufs=2) as stat,
            tc.tile_pool(name="constp", bufs=1) as constp,
            tc.tile_pool(name="epool", bufs=1, space="PSUM") as epool,
            tc.tile_pool(name="tppool", bufs=2, space="PSUM") as tppool,
            tc.tile_pool(name="pcpool", bufs=2, space="PSUM") as pcpool,
        ):
            # ---- constants (param DMAs go on the ACT engine's queues so
            # they never delay the first x loads on SP's queues) ----
            ident = constp.tile([128, 128], f32, name="ident")
            make_identity(nc, ident)
            ident_b = constp.tile([128, 128], bf16, name="identb")
            nc.vector.tensor_copy(ident_b, ident)
            # scratch dest for ACT copy-with-accum row sums (value unused)
            actdump = constp.tile([128, HW], bf16, name="actdump")

            def emit_params():
                w1s = []
                for k in range(CT):
                    w1raw = constp.tile([128, R], f32, name=f"w1raw{k}")
                    nc.scalar.dma_start(w1raw, w1_d[128 * k:128 * (k + 1), :])
                    w1k = constp.tile([128, R], f32, name=f"w1s{k}")
                    # fold the 1/HW of the global average pool into W1
                    nc.vector.tensor_scalar_mul(w1k, w1raw, 1.0 / HW)
                    w1s.append(w1k)

                w2_sb = constp.tile([R, C], f32, name="w2sb")
                nc.scalar.dma_start(w2_sb, w2_d)
                b1_sb = constp.tile([R, 1], f32, name="b1sb")
                nc.scalar.dma_start(b1_sb, b1_d)
                negb2 = []
                for m in range(CT):
                    b2raw = constp.tile([128, 1], f32, name=f"b2raw{m}")
                    nc.scalar.dma_start(b2raw, b2_d[128 * m:128 * (m + 1), :])
                    nb2 = constp.tile([128, 1], f32, name=f"negb2{m}")
                    nc.vector.tensor_scalar_mul(nb2, b2raw, -1.0)
                    negb2.append(nb2)

                g_sb = constp.tile([1, 1], f32, name="gsb")
                nc.scalar.dma_start(g_sb, g_d)
                g128 = constp.tile([128, 1], f32, name="g128")
                nc.gpsimd.partition_broadcast(g128, g_sb[0:1, :])
                return w1s, w2_sb, b1_sb, negb2, g128

            params = None

            def emit_load(s, fine_first):
                """DMA one sample's x into f32 tiles + GPSIMD bf16 cast.

                GPSIMD is otherwise idle and streams 1-input copies at line
                rate, so the bf16 attention copy costs no DVE/ACT/PE time
                and no extra HBM traffic.
                """
                def cast_eng_for(cch):
                    return nc.gpsimd
                q, qb = [], []
                for i in range(CT):
                    q_i = qpool.tile([128, HW], f32, name=f"q{i}", tag=f"q{i}")
                    q.append(q_i)
                    qb_i = qpool.tile(
                        [128, HW], bf16, name=f"qb{i}", tag=f"qb{i}",
                        bufs=CFG["qb_bufs"],
                    )
                    qb.append(qb_i)
                for cch in range(8):
                    csl = slice(512 * cch, 512 * (cch + 1))
                    for i in range(CT):
                        nc.sync.dma_start(
                            q[i][:, csl], x_d[s, 128 * i:128 * (i + 1), csl]
                        )
                        cast_eng_for(cch).tensor_copy(
                            qb[i][:, csl], q[i][:, csl]
                        )
                return q, qb

            loaded = {0: emit_load(0, False)}

            for s in range(BS):
                q, qb = loaded.pop(s)
                if params is None:
                    params = emit_params()
                w1s, w2_sb, b1_sb, negb2, g128 = params

                # ---- SE row sums of x ----
                # sample 0: chunked DVE partial reduces in the early idle
                # window (emitted here, before MM1).
                scol = []
                if s == 0:
                    for m in range(CT):
                        part = stat.tile(
                            [128, 4], f32, name=f"spart{m}", tag=f"spart{m}"
                        )
                        for j in range(4):
                            nc.vector.tensor_reduce(
                                part[:, j:j + 1],
                                q[m][:, 1024 * j:1024 * (j + 1)],
                                axis=mybir.AxisListType.X,
                                op=ALU.add,
                            )
                        sc = stat.tile(
                            [128, 1], f32, name=f"scol{m}", tag=f"scol{m}"
                        )
                        nc.vector.tensor_reduce(
                            sc, part, axis=mybir.AxisListType.X, op=ALU.add
                        )
                        scol.append(sc)
                else:
                    for m in range(CT):
                        sc = stat.tile(
                            [128, 1], f32, name=f"scol{m}", tag=f"scol{m}"
                        )
                        nc.scalar.activation(
                            actdump, q[m], ACT.Copy, accum_out=sc
                        )
                        scol.append(sc)

                # ---- transpose q -> qT, pipelined with MM1 accumulation ----
                e_ps = [
                    epool.tile([128, 512], f32, name=f"e{m}", tag=f"e{m}")
                    for m in range(CT)
                ]

                def emit_trans(t):
                    tp = tppool.tile([128, 512], bf16, name="tp", tag="tp")
                    for i in range(CT):
                        nc.tensor.transpose(
                            tp[:, 128 * i:128 * (i + 1)],
                            qb[i][:, 128 * t:128 * (t + 1)],
                            ident_b,
                        )
                    qT = qtpool.tile([128, 512], bf16, name="qT", tag="qT")
                    nc.scalar.copy(qT, tp)
                    return qT

                # energy is symmetric: compute only d >= 128*m per row-tile
                SPLIT = NT - 4
                qTs = {}
                pend = emit_trans(0)
                for t in range(SPLIT):
                    cur = pend
                    pend = emit_trans(t + 1)
                    for m in range(CT):
                        nc.tensor.matmul(
                            e_ps[m][:, 128 * m:],
                            cur[:, 128 * m:128 * (m + 1)],
                            cur[:, 128 * m:],
                            start=(t == 0),
                            stop=False,
                        )
                qTs[SPLIT] = pend
                for t in range(SPLIT + 1, NT):
                    qTs[t] = emit_trans(t)
                for m in range(CT):
                    for t in range(SPLIT, NT):
                        nc.tensor.matmul(
                            e_ps[m][:, 128 * m:],
                            qTs[t][:, 128 * m:128 * (m + 1)],
                            qTs[t][:, 128 * m:],
                            start=False,
                            stop=(t == NT - 1),
                        )

                # ---- softmax ----
                # Upper blocks read energy directly; lower blocks [i][:, j<i]
                # are exp(mn_i - E[j][:, i].T) via a PSUM->SBUF copy + PE
                # transpose of the symmetric partner block. The stabilizer
                # need only be a per-row upper bound on -e, and softmax
                # cancels any per-row constant, so bf16 block copies are
                # safe.
                ebs = {}   # (j, i) -> transposed-energy block (SBUF)
                for i in range(CT):
                    for j in range(i):
                        eb = stat.tile(
                            [128, 128], bf16, name=f"eb{j}{i}",
                            tag=f"eb{j}{i}", bufs=1,
                        )
                        nc.scalar.copy(eb, e_ps[j][:, 128 * i:128 * (i + 1)])
                        tb = tppool.tile(
                            [128, 128], bf16, name="tb", tag="tp"
                        )
                        nc.tensor.transpose(tb, eb, ident_b)
                        # evacuate to SBUF immediately so the PSUM ring slot
                        # frees without waiting on the downstream mn/exp chain
                        ebT = stat.tile(
                            [128, 128], bf16, name=f"ebT{j}{i}",
                            tag=f"ebT{j}{i}", bufs=1,
                        )
                        nc.vector.tensor_copy(ebT, tb)
                        ebs[(j, i)] = ebT

                Ps, rZ = [], []
                for i in range(CT):
                    mns = []
                    mn0 = stat.tile([128, 1], f32, name=f"mn{i}", tag=f"mn{i}")
                    nc.vector.tensor_reduce(
                        mn0, e_ps[i][:, 128 * i:],
                        axis=mybir.AxisListType.X, op=ALU.min,
                    )
                    mns.append(mn0)
                    for j in range(i):
                        bmn = stat.tile(
                            [128, 1], f32, name=f"bmn{i}{j}", tag=f"bmn{i}{j}"
                        )
                        nc.vector.tensor_reduce(
                            bmn, ebs[(j, i)],
                            axis=mybir.AxisListType.X, op=ALU.min,
                        )
                        mns.append(bmn)
                    mn = mns[0]
                    for v, bmn in enumerate(mns[1:]):
                        mn2 = stat.tile(
                            [128, 1], f32, name=f"mnc{i}{v}", tag=f"mnc{i}{v}"
                        )
                        nc.vector.tensor_tensor(mn2, mn, bmn, op=ALU.min)
                        mn = mn2
                    P_m = ppool.tile([128, 512], bf16, name=f"P{i}", tag=f"P{i}")
                    Zs = []
                    Zt = stat.tile([128, 1], f32, name=f"Z{i}", tag=f"Z{i}")
                    nc.scalar.activation(
                        P_m[:, 128 * i:], e_ps[i][:, 128 * i:], ACT.Exp,
                        bias=mn, scale=-1.0, accum_out=Zt,
                    )
                    Zs.append(Zt)
                    for j in range(i):
                        Zb = stat.tile(
                            [128, 1], f32, name=f"Zb{i}{j}", tag=f"Zb{i}{j}"
                        )
                        nc.scalar.activation(
                            P_m[:, 128 * j:128 * (j + 1)], ebs[(j, i)],
                            ACT.Exp, bias=mn, scale=-1.0, accum_out=Zb,
                        )
                        Zs.append(Zb)
                    Z = Zs[0]
                    for v, Zb in enumerate(Zs[1:]):
                        Z2 = stat.tile(
                            [128, 1], f32, name=f"Zc{i}{v}", tag=f"Zc{i}{v}"
                        )
                        nc.vector.tensor_add(Z2, Z, Zb)
                        Z = Z2
                    rz = stat.tile([128, 1], f32, name=f"rz{i}", tag=f"rz{i}")
                    nc.vector.reciprocal(rz, Z)
                    Ps.append(P_m)
                    rZ.append(rz)

                # ---- SE MLP in column layout (all f32, tiny) ----
                hp = pcpool.tile([64, 1], f32, name="hp", tag="pc")
                for k in range(CT):
                    nc.tensor.matmul(
                        hp,
                        w1s[k],
                        scol[k],
                        start=(k == 0),
                        stop=(k == CT - 1),
                    )
                h = stat.tile([64, 1], f32, name="h", tag="h")
                nc.scalar.activation(h, hp, ACT.Relu, bias=b1_sb, scale=1.0)

                alph = []
                for m in range(CT):
                    sp = pcpool.tile([128, 1], f32, name=f"sp{m}", tag="pc")
                    nc.tensor.matmul(
                        sp,
                        w2_sb[:, 128 * m:128 * (m + 1)],
                        h,
                    )
                    # sigmoid(v) = 1 / (1 + exp(-v)); stays in the exp table set
                    u = stat.tile([128, 1], f32, name=f"u{m}", tag=f"u{m}")
                    nc.scalar.activation(
                        u, sp, ACT.Exp, bias=negb2[m], scale=-1.0
                    )
                    t1 = stat.tile([128, 1], f32, name=f"t1{m}", tag=f"t1{m}")
                    nc.vector.tensor_scalar_add(t1, u, 1.0)
                    sig = stat.tile([128, 1], f32, name=f"sig{m}", tag=f"sig{m}")
                    nc.vector.reciprocal(sig, t1)
                    a1 = stat.tile([128, 1], f32, name=f"a1{m}", tag=f"a1{m}")
                    nc.vector.tensor_mul(a1, sig, rZ[m])
                    a2 = stat.tile([128, 1], f32, name=f"a2{m}", tag=f"a2{m}")
                    nc.vector.tensor_mul(a2, a1, g128)
                    alph.append(a2)

                # ---- transpose P -> PT ----
                # ptp tiles reuse the (now dead) energy PSUM banks; i-major
                # order lets transposes of P_i start as soon as exp(i) lands.
                ptps = [
                    epool.tile([128, 512], bf16, name=f"ptp{j}", tag=f"e{j}")
                    for j in range(CT)
                ]
                for i in range(CT):
                    for j in range(CT):
                        nc.tensor.transpose(
                            ptps[j][:, 128 * i:128 * (i + 1)],
                            Ps[i][:, 128 * j:128 * (j + 1)],
                            ident_b,
                        )
                PTs = []
                for j in range(CT):
                    PT_j = ptpool.tile(
                        [128, 512], bf16, name=f"PT{j}", tag=f"PT{j}"
                    )
                    nc.vector.tensor_copy(PT_j, ptps[j])
                    PTs.append(PT_j)

                # prefetch next sample's x during this sample's MM2 so the
                # SP DMA triggers aren't stuck behind data-gated out-DMAs
                if s + 1 < BS:
                    loaded[s + 1] = emit_load(s + 1, False)

                # ---- matmul2 + fused scale/residual + store ----
                for m in range(CT):
                    for ch in range(NCH):
                        nsl = slice(512 * ch, 512 * (ch + 1))
                        pc = pcpool.tile([128, 512], f32, name="pc", tag="pc")
                        for k in range(CT):
                            nc.tensor.matmul(
                                pc,
                                PTs[k][:, 128 * m:128 * (m + 1)],
                                qb[k][:, nsl],
                                start=(k == 0),
                                stop=(k == CT - 1),
                            )
                        st = stpool.tile(
                            [128, 512], f32, name="st", tag="st",
                            bufs=CFG["st_bufs"],
                        )
                        nc.vector.scalar_tensor_tensor(
                            st, pc, alph[m], q[m][:, nsl],
                            op0=ALU.mult, op1=ALU.add,
                        )
                        out_eng = getattr(nc, {"sync": "sync", "scalar": "scalar"}[CFG["out_eng"]])
                        out_eng.dma_start(
                            out_d[s, 128 * m:128 * (m + 1), nsl], st
                        )

    nc.compile()
    _BUILT = nc
    return nc


def kernel(**inputs):
    global LAST_RESULTS
    from concourse.bass_utils import run_bass_kernel_spmd

    x = np.ascontiguousarray(np.asarray(inputs["x"], dtype=np.float32))
    gamma = np.asarray(inputs["gamma"], dtype=np.float32)
    W1 = np.ascontiguousarray(np.asarray(inputs["W1"], dtype=np.float32))
    b1 = np.asarray(inputs["b1"], dtype=np.float32)
    W2 = np.ascontiguousarray(np.asarray(inputs["W2"], dtype=np.float32))
    b2 = np.asarray(inputs["b2"], dtype=np.float32)

    nc = _build()

    xr = x.reshape(B, C, HW)
    b1c = np.ascontiguousarray(b1.reshape(R, 1))
    b2c = np.ascontiguousarray(b2.reshape(C, 1))
    gc = np.ascontiguousarray(gamma.reshape(1, 1))

    in_maps = []
    for c in range(NCORES):
        shard = np.ascontiguousarray(xr[BS * c: BS * (c + 1)])
        in_maps.append(
            {"x": shard, "w1": W1, "b1": b1c, "w2": W2,
             "b2": b2c, "gam": gc}
        )

    res = run_bass_kernel_spmd(
        nc, in_maps, core_ids=list(range(NCORES)), trace=TRACE
    )
    LAST_RESULTS = res

    out = np.concatenate([r["out"] for r in res.results], axis=0)
    return out.reshape(B, C, H, W).astype(np.float32, copy=False)
